# revision 11
# baseline (speedup 1.0000x reference)
"""Conv3d(32->64, k=3, pad=1) + BatchNorm(training) + LeakyReLU(0.2) on
(2, 32, 96, 96, 35), distributed over 8 TRN2 NeuronCores.

Strategy:
  - Shard H (96 = 8 x 12 rows per core). Halo rows + spatial zero-padding are
    materialized host-side into a per-core tensor xs of shape (2,32,14,98,39)
    (1 zero row each side of the 12-row H shard; W padded 96->98; D padded
    35->39 so that three d-shifted SBUF copies can be loaded by shifted reads).
  - Conv as implicit GEMM in bf16 (f32 PSUM accumulate): contraction
    K = 96 = C_in(32) x kd(3).  The SBUF "slab" for one input row holds 3
    partition-groups, group j pre-shifted by j elements along D.  Each of the
    9 (kh,kw) taps is then one matmul whose rhs is a free-dim-shifted window
    of the slab; kd is folded into the contraction.  PSUM accumulates taps.
  - M = C_out = 64 uses half the PE columns, so two spatial w-tiles run as a
    column-pair: tile A -> psum[0:64], tile B -> psum[64:128] (distinct PE
    column groups overlap in hardware).
  - W is tiled 96 = 8 x 12; a matmul streams 12w x 37d = 444 columns (the 2
    padded d columns per w are computed but never evicted).
  - BatchNorm (training stats): bn_stats per evicted tile -> bn_aggr ->
    per-partition (mean, var) -> (sum, sumsq) -> tiny AllReduce over the 8
    cores -> scale/shift folded into one parametric-relu activation.  The
    conv bias b cancels exactly in training-mode BN and is unused.  Conv
    values are parked in SBUF as bf16.
  - DMA: input slab loads ride the SP HWDGE ring, output stores the ACT
    HWDGE ring, so transfers overlap.
"""

import numpy as np
import ml_dtypes

import concourse.bacc as bacc
import concourse.bass as bass
import concourse.tile as tile
from concourse import mybir
from concourse.bass_utils import run_bass_kernel_spmd

N_CORES = 8
B, C_IN, C_OUT = 2, 32, 64
H, W, D = 96, 96, 35
HS = H // N_CORES          # 12 output rows per core
HR = HS + 2                # 14 input rows (halo)
WP, DP = W + 2, D + 2      # padded W / padded D for the host tensor
RW = D + 2                 # 37: slab row width per w-column (full padded D)
SLAB = WP * RW + 2 + 30    # slab row extent incl. group-shift + junk-read slack
WT = 12                    # w-tile width (8 uniform tiles)
NFULL = WT * RW            # 468 matmul free size
EVF = WT * D               # 420 evicted columns per tile
BLK = B * HS               # 24 (b,h) blocks per core
BLKCOLS = 4 * EVF          # 1680 conv-buffer columns per block per half
NREC = BLK * 4             # 96 bn_stats records per partition
CNT = float(BLK * 4 * EVF)          # elements per partition (40320)
N_TOT = float(B * H * W * D)        # 645120
EPS = 1e-5
NEG = 0.2

F32 = mybir.dt.float32
BF16 = mybir.dt.bfloat16
NP_BF16 = ml_dtypes.bfloat16

_CACHE = {}


def _build():
    nc = bacc.Bacc("TRN2", target_bir_lowering=False, debug=False,
                   num_devices=N_CORES)
    xs = nc.dram_tensor("xs", [B, C_IN, HR, WP, DP], BF16, kind="ExternalInput")
    wt = nc.dram_tensor("wt", [3, 3, 96, C_OUT], BF16, kind="ExternalInput")
    gm = nc.dram_tensor("gm", [C_OUT], F32, kind="ExternalInput")
    bt = nc.dram_tensor("bt", [C_OUT], F32, kind="ExternalInput")
    ys = nc.dram_tensor("ys", [B, C_OUT, HS, W, D], F32, kind="ExternalOutput")

    xs_ap = xs.ap()
    ys_ap = ys.ap()

    with tile.TileContext(nc) as tc:
        with tc.tile_pool(name="singles", bufs=1) as singles, \
             tc.tile_pool(name="slab", bufs=4) as slabp, \
             tc.tile_pool(name="psum", bufs=4, space="PSUM") as psump, \
             tc.tile_pool(name="stg", bufs=4) as stgp, \
             tc.tile_pool(name="dram", bufs=1, space="DRAM") as dramp:

            # ---- one-time loads ----
            wtile = singles.tile([96, 9, C_OUT], BF16)
            nc.sync.dma_start(
                out=wtile,
                in_=wt.ap().rearrange("kh kw p o -> p (kh kw) o"))
            gmt = singles.tile([C_OUT, 1], F32)
            nc.sync.dma_start(out=gmt, in_=gm.ap().rearrange("(p o) -> p o", o=1))
            btt = singles.tile([C_OUT, 1], F32)
            nc.sync.dma_start(out=btt, in_=bt.ap().rearrange("(p o) -> p o", o=1))

            cb = singles.tile([128, BLK * BLKCOLS], BF16)   # conv results
            st = singles.tile([128, NREC * 6], F32)          # bn_stats records

            # ---- pass 1: conv + stats ----
            # slab group tiles hold 2 input rows (rows 2g, 2g+1)
            for b in range(B):
                groups = {}

                def need_rows(lo, hi, b=b, groups=None):
                    pass

                for h in range(HS):
                    for r in (h, h + 1, h + 2):
                        g = r // 2
                        if g not in groups:
                            gt = slabp.tile([96, 2, SLAB], BF16, tag="slab")
                            for j in range(3):
                                # group j holds the full row shifted by (2-j)
                                nc.sync.dma_start(
                                    out=gt[32 * j:32 * (j + 1), :,
                                           2 - j:2 - j + WP * RW],
                                    in_=xs_ap[b, :, 2 * g:2 * g + 2, :, :].rearrange(
                                        "p r w d -> p r (w d)"))
                            groups[g] = gt
                    blk = b * HS + h
                    for k in range(4):
                        w0a = 2 * k * WT
                        w0b = (2 * k + 1) * WT
                        ps = psump.tile([128, NFULL], F32, tag="ps")
                        for kh in range(3):
                            r = h + kh
                            gt = groups[r // 2]
                            rs = r % 2
                            for kw in range(3):
                                q = kh * 3 + kw
                                first, last = q == 0, q == 8
                                oa = (w0a + kw) * RW + 2
                                ob = (w0b + kw) * RW + 2
                                nc.tensor.matmul(
                                    ps[0:64, :],
                                    lhsT=wtile[:, q, :],
                                    rhs=gt[:, rs, oa:oa + NFULL],
                                    start=first, stop=last)
                                nc.tensor.matmul(
                                    ps[64:128, :],
                                    lhsT=wtile[:, q, :],
                                    rhs=gt[:, rs, ob:ob + NFULL],
                                    start=first, stop=last)
                        # evict + stats
                        col = blk * BLKCOLS + k * EVF
                        rec = (blk * 4 + k) * 6
                        pv = ps.rearrange("p (w d) -> p w d", d=RW)[:, :, 0:D]
                        nc.scalar.copy(
                            out=cb[:, col:col + EVF].rearrange(
                                "p (w d) -> p w d", d=D),
                            in_=pv)
                        nc.vector.bn_stats(out=st[:, rec:rec + 6],
                                           in_=cb[:, col:col + EVF])

            # ---- stats aggregation + allreduce ----
            mv = singles.tile([128, 2], F32)
            nc.vector.bn_aggr(out=mv, in_=st.rearrange("p (r s) -> p r s", s=6))
            sq = singles.tile([128, 2], F32)
            t1 = singles.tile([128, 1], F32)
            # sum = mean * n ; sumsq = (var + mean^2) * n
            nc.vector.tensor_scalar_mul(sq[:, 0:1], mv[:, 0:1], CNT)
            nc.vector.tensor_mul(t1, mv[:, 0:1], mv[:, 0:1])
            nc.vector.tensor_add(t1, t1, mv[:, 1:2])
            nc.vector.tensor_scalar_mul(sq[:, 1:2], t1, CNT)

            cc_in = dramp.tile([128, 2], F32)
            cc_out = dramp.tile([128, 2], F32)
            nc.sync.dma_start(out=cc_in[:, :], in_=sq)
            nc.gpsimd.collective_compute(
                "AllReduce", mybir.AluOpType.add,
                replica_groups=[list(range(N_CORES))],
                ins=[cc_in[:, :].opt()], outs=[cc_out[:, :].opt()])
            gl = singles.tile([128, 2], F32)
            nc.sync.dma_start(out=gl, in_=cc_out[:, :])

            hi = singles.tile([64, 2], F32)
            nc.sync.dma_start(out=hi, in_=gl[64:128, :])
            tot = singles.tile([64, 2], F32)
            nc.vector.tensor_add(tot, gl[0:64, :], hi)

            m_g = singles.tile([64, 1], F32)
            qn = singles.tile([64, 1], F32)
            var = singles.tile([64, 1], F32)
            sd = singles.tile([64, 1], F32)
            s64 = singles.tile([64, 1], F32)
            t64 = singles.tile([64, 1], F32)
            nc.vector.tensor_scalar_mul(m_g, tot[:, 0:1], 1.0 / N_TOT)
            nc.vector.tensor_scalar_mul(qn, tot[:, 1:2], 1.0 / N_TOT)
            nc.vector.tensor_mul(var, m_g, m_g)
            nc.vector.tensor_sub(var, qn, var)
            epst = singles.tile([64, 1], F32)
            nc.vector.memset(epst, EPS)
            nc.scalar.activation(out=sd, in_=var,
                                 func=mybir.ActivationFunctionType.Sqrt,
                                 bias=epst)
            nc.vector.reciprocal(out=sd, in_=sd)
            nc.vector.tensor_mul(s64, sd, gmt)      # s = gamma * rsqrt(var+eps)
            nc.vector.tensor_mul(t64, m_g, s64)
            nc.vector.tensor_sub(t64, btt, t64)     # t = beta - mean * s

            s_all = singles.tile([128, 1], F32)
            t_all = singles.tile([128, 1], F32)
            nc.vector.tensor_copy(s_all[0:64, :], s64)
            nc.vector.tensor_copy(t_all[0:64, :], t64)
            nc.sync.dma_start(out=s_all[64:128, :], in_=s_all[0:64, :])
            nc.sync.dma_start(out=t_all[64:128, :], in_=t_all[0:64, :])

            # ---- pass 2: normalize + LeakyReLU + writeback ----
            c_step = HS * W * D  # ys channel stride
            for blk in range(0, BLK, 2):
                b_, h_ = divmod(blk, HS)
                stg = stgp.tile([128, 2 * BLKCOLS], F32, tag="stg")
                nc.scalar.activation(
                    out=stg, in_=cb[:, blk * BLKCOLS:(blk + 2) * BLKCOLS],
                    func=mybir.ActivationFunctionType.Prelu,
                    bias=t_all, scale=s_all, alpha=NEG)
                base_off = ys_ap.offset + b_ * (C_OUT * c_step) + h_ * (W * D)
                # two consecutive h rows; w-tiles at w = 0,24,48,72 (A) / +12 (B)
                dst_a = bass.AP(
                    tensor=ys_ap.tensor, offset=base_off,
                    ap=[[c_step, C_OUT], [W * D, 2], [2 * WT * D, 4],
                        [D, WT], [1, D]])
                nc.sync.dma_start(
                    out=dst_a,
                    in_=stg[0:64, :].rearrange(
                        "p (r t w d) -> p r t w d", r=2, t=4, d=D))
                dst_b = bass.AP(
                    tensor=ys_ap.tensor, offset=base_off + WT * D,
                    ap=[[c_step, C_OUT], [W * D, 2], [2 * WT * D, 4],
                        [D, WT], [1, D]])
                nc.gpsimd.dma_start(
                    out=dst_b,
                    in_=stg[64:128, :].rearrange(
                        "p (r t w d) -> p r t w d", r=2, t=4, d=D))

    nc.finalize()
    return nc


def _get_nc():
    if "nc" not in _CACHE:
        _CACHE["nc"] = _build()
    return _CACHE["nc"]


def _prep(x, w, gamma, beta):
    xpad = np.zeros((B, C_IN, H + 2, WP, DP), dtype=np.float32)
    xpad[:, :, 1:H + 1, 1:W + 1, 1:D + 1] = x
    wt = np.ascontiguousarray(
        np.asarray(w, dtype=np.float32).transpose(2, 3, 4, 1, 0).reshape(
            3, 3, 96, C_OUT)).astype(NP_BF16)
    gm = np.ascontiguousarray(np.asarray(gamma, dtype=np.float32))
    bt = np.ascontiguousarray(np.asarray(beta, dtype=np.float32))
    in_maps = []
    for c in range(N_CORES):
        xsl = np.ascontiguousarray(
            xpad[:, :, c * HS:c * HS + HR, :, :]).astype(NP_BF16)
        in_maps.append({"xs": xsl, "wt": wt, "gm": gm, "bt": bt})
    return in_maps


def kernel(x, w, b, gamma, beta):
    nc = _get_nc()
    in_maps = _prep(np.asarray(x, dtype=np.float32), w, gamma, beta)
    res = run_bass_kernel_spmd(nc, in_maps, core_ids=list(range(N_CORES)))
    out = np.concatenate([res.results[c]["ys"] for c in range(N_CORES)], axis=2)
    return out.astype(np.float32)


# revision 12
# speedup vs baseline: 1.0973x; 1.0973x over previous
"""Conv3d(32->64, k=3, pad=1) + BatchNorm(training) + LeakyReLU(0.2) on
(2, 32, 96, 96, 35), distributed over 8 TRN2 NeuronCores.

Strategy:
  - Shard H (96 = 8 x 12 rows per core). Halo rows + spatial zero-padding are
    materialized host-side into a per-core tensor xs of shape (2,32,14,98,39)
    (1 zero row each side of the 12-row H shard; W padded 96->98; D padded
    35->39 so that three d-shifted SBUF copies can be loaded by shifted reads).
  - Conv as implicit GEMM in bf16 (f32 PSUM accumulate): contraction
    K = 96 = C_in(32) x kd(3).  The SBUF "slab" for one input row holds 3
    partition-groups, group j pre-shifted by j elements along D.  Each of the
    9 (kh,kw) taps is then one matmul whose rhs is a free-dim-shifted window
    of the slab; kd is folded into the contraction.  PSUM accumulates taps.
  - M = C_out = 64 uses half the PE columns, so two spatial w-tiles run as a
    column-pair: tile A -> psum[0:64], tile B -> psum[64:128] (distinct PE
    column groups overlap in hardware).
  - W is tiled 96 = 8 x 12; a matmul streams 12w x 37d = 444 columns (the 2
    padded d columns per w are computed but never evicted).
  - BatchNorm (training stats): bn_stats per evicted tile -> bn_aggr ->
    per-partition (mean, var) -> (sum, sumsq) -> tiny AllReduce over the 8
    cores -> scale/shift folded into one parametric-relu activation.  The
    conv bias b cancels exactly in training-mode BN and is unused.  Conv
    values are parked in SBUF as bf16.
  - DMA: input slab loads ride the SP HWDGE ring, output stores the ACT
    HWDGE ring, so transfers overlap.
"""

import numpy as np
import ml_dtypes

import concourse.bacc as bacc
import concourse.bass as bass
import concourse.tile as tile
from concourse import mybir
from concourse.bass_utils import run_bass_kernel_spmd

N_CORES = 8
B, C_IN, C_OUT = 2, 32, 64
H, W, D = 96, 96, 35
HS = H // N_CORES          # 12 output rows per core
HR = HS + 2                # 14 input rows (halo)
WP, DP = W + 2, D + 2      # padded W / padded D for the host tensor
RW = D + 2                 # 37: slab row width per w-column (full padded D)
SLAB = WP * RW + 2 + 30    # slab row extent incl. group-shift + junk-read slack
WT = 12                    # w-tile width (8 uniform tiles)
NFULL = WT * RW            # 468 matmul free size
EVF = WT * D               # 420 evicted columns per tile
BLK = B * HS               # 24 (b,h) blocks per core
BLKCOLS = 4 * EVF          # 1680 conv-buffer columns per block per half
NREC = BLK * 4             # 96 bn_stats records per partition
CNT = float(BLK * 4 * EVF)          # elements per partition (40320)
N_TOT = float(B * H * W * D)        # 645120
EPS = 1e-5
NEG = 0.2

F32 = mybir.dt.float32
BF16 = mybir.dt.bfloat16
NP_BF16 = ml_dtypes.bfloat16

_CACHE = {}


def _build():
    nc = bacc.Bacc("TRN2", target_bir_lowering=False, debug=False,
                   num_devices=N_CORES)
    xs = nc.dram_tensor("xs", [B, C_IN, HR, WP, DP], BF16, kind="ExternalInput")
    wt = nc.dram_tensor("wt", [3, 3, 96, C_OUT], BF16, kind="ExternalInput")
    gm = nc.dram_tensor("gm", [C_OUT], F32, kind="ExternalInput")
    bt = nc.dram_tensor("bt", [C_OUT], F32, kind="ExternalInput")
    ys = nc.dram_tensor("ys", [B, C_OUT, HS, W, D], F32, kind="ExternalOutput")

    xs_ap = xs.ap()
    ys_ap = ys.ap()

    with tile.TileContext(nc) as tc:
        with tc.tile_pool(name="singles", bufs=1) as singles, \
             tc.tile_pool(name="slab", bufs=4) as slabp, \
             tc.tile_pool(name="psum", bufs=4, space="PSUM") as psump, \
             tc.tile_pool(name="stg", bufs=4) as stgp, \
             tc.tile_pool(name="dram", bufs=1, space="DRAM") as dramp:

            # ---- one-time loads ----
            wtile = singles.tile([96, 9, C_OUT], BF16)
            nc.sync.dma_start(
                out=wtile,
                in_=wt.ap().rearrange("kh kw p o -> p (kh kw) o"))
            gmt = singles.tile([C_OUT, 1], F32)
            nc.sync.dma_start(out=gmt, in_=gm.ap().rearrange("(p o) -> p o", o=1))
            btt = singles.tile([C_OUT, 1], F32)
            nc.sync.dma_start(out=btt, in_=bt.ap().rearrange("(p o) -> p o", o=1))

            cb = singles.tile([128, BLK * BLKCOLS], BF16)   # conv results
            st = singles.tile([128, NREC * 6], F32)          # bn_stats records

            # ---- pass 1: conv + stats ----
            # slab group tiles hold 2 input rows (rows 2g, 2g+1)
            for b in range(B):
                groups = {}

                def need_rows(lo, hi, b=b, groups=None):
                    pass

                for h in range(HS):
                    for r in (h, h + 1, h + 2):
                        g = r // 2
                        if g not in groups:
                            gt = slabp.tile([96, 2, SLAB], BF16, tag="slab")
                            for j in range(3):
                                # group j holds the full row shifted by (2-j)
                                nc.sync.dma_start(
                                    out=gt[32 * j:32 * (j + 1), :,
                                           2 - j:2 - j + WP * RW],
                                    in_=xs_ap[b, :, 2 * g:2 * g + 2, :, :].rearrange(
                                        "p r w d -> p r (w d)"))
                            groups[g] = gt
                    blk = b * HS + h
                    for k in range(4):
                        w0a = 2 * k * WT
                        w0b = (2 * k + 1) * WT
                        ps = psump.tile([128, NFULL], F32, tag="ps")
                        for kh in range(3):
                            r = h + kh
                            gt = groups[r // 2]
                            rs = r % 2
                            for kw in range(3):
                                q = kh * 3 + kw
                                first, last = q == 0, q == 8
                                oa = (w0a + kw) * RW + 2
                                ob = (w0b + kw) * RW + 2
                                nc.tensor.matmul(
                                    ps[0:64, :],
                                    lhsT=wtile[:, q, :],
                                    rhs=gt[:, rs, oa:oa + NFULL],
                                    start=first, stop=last)
                                nc.tensor.matmul(
                                    ps[64:128, :],
                                    lhsT=wtile[:, q, :],
                                    rhs=gt[:, rs, ob:ob + NFULL],
                                    start=first, stop=last)
                        # evict + stats
                        col = blk * BLKCOLS + k * EVF
                        rec = (blk * 4 + k) * 6
                        pv = ps.rearrange("p (w d) -> p w d", d=RW)[:, :, 0:D]
                        nc.scalar.copy(
                            out=cb[:, col:col + EVF].rearrange(
                                "p (w d) -> p w d", d=D),
                            in_=pv)
                        nc.vector.bn_stats(out=st[:, rec:rec + 6],
                                           in_=cb[:, col:col + EVF])

            # ---- stats aggregation + allreduce ----
            mv = singles.tile([128, 2], F32)
            nc.vector.bn_aggr(out=mv, in_=st.rearrange("p (r s) -> p r s", s=6))
            sq = singles.tile([128, 2], F32)
            t1 = singles.tile([128, 1], F32)
            # sum = mean * n ; sumsq = (var + mean^2) * n
            nc.vector.tensor_scalar_mul(sq[:, 0:1], mv[:, 0:1], CNT)
            nc.vector.tensor_mul(t1, mv[:, 0:1], mv[:, 0:1])
            nc.vector.tensor_add(t1, t1, mv[:, 1:2])
            nc.vector.tensor_scalar_mul(sq[:, 1:2], t1, CNT)

            cc_in = dramp.tile([128, 2], F32)
            cc_out = dramp.tile([128, 2], F32)
            nc.sync.dma_start(out=cc_in[:, :], in_=sq)
            nc.gpsimd.collective_compute(
                "AllReduce", mybir.AluOpType.add,
                replica_groups=[list(range(N_CORES))],
                ins=[cc_in[:, :].opt()], outs=[cc_out[:, :].opt()])
            gl = singles.tile([128, 2], F32)
            nc.sync.dma_start(out=gl, in_=cc_out[:, :])

            hi = singles.tile([64, 2], F32)
            nc.sync.dma_start(out=hi, in_=gl[64:128, :])
            tot = singles.tile([64, 2], F32)
            nc.vector.tensor_add(tot, gl[0:64, :], hi)

            m_g = singles.tile([64, 1], F32)
            qn = singles.tile([64, 1], F32)
            var = singles.tile([64, 1], F32)
            sd = singles.tile([64, 1], F32)
            s64 = singles.tile([64, 1], F32)
            t64 = singles.tile([64, 1], F32)
            nc.vector.tensor_scalar_mul(m_g, tot[:, 0:1], 1.0 / N_TOT)
            nc.vector.tensor_scalar_mul(qn, tot[:, 1:2], 1.0 / N_TOT)
            nc.vector.tensor_mul(var, m_g, m_g)
            nc.vector.tensor_sub(var, qn, var)
            epst = singles.tile([64, 1], F32)
            nc.vector.memset(epst, EPS)
            nc.scalar.activation(out=sd, in_=var,
                                 func=mybir.ActivationFunctionType.Sqrt,
                                 bias=epst)
            nc.vector.reciprocal(out=sd, in_=sd)
            nc.vector.tensor_mul(s64, sd, gmt)      # s = gamma * rsqrt(var+eps)
            nc.vector.tensor_mul(t64, m_g, s64)
            nc.vector.tensor_sub(t64, btt, t64)     # t = beta - mean * s

            s_all = singles.tile([128, 1], F32)
            t_all = singles.tile([128, 1], F32)
            nc.vector.tensor_copy(s_all[0:64, :], s64)
            nc.vector.tensor_copy(t_all[0:64, :], t64)
            nc.sync.dma_start(out=s_all[64:128, :], in_=s_all[0:64, :])
            nc.sync.dma_start(out=t_all[64:128, :], in_=t_all[0:64, :])

            # ---- pass 2: normalize + LeakyReLU + writeback ----
            c_step = HS * W * D  # ys channel stride
            for blk in range(0, BLK, 2):
                b_, h_ = divmod(blk, HS)
                stg = stgp.tile([128, 2 * BLKCOLS], F32, tag="stg")
                nc.scalar.activation(
                    out=stg, in_=cb[:, blk * BLKCOLS:(blk + 2) * BLKCOLS],
                    func=mybir.ActivationFunctionType.Prelu,
                    bias=t_all, scale=s_all, alpha=NEG)
                base_off = ys_ap.offset + b_ * (C_OUT * c_step) + h_ * (W * D)
                # two consecutive h rows; w-tiles at w = 0,24,48,72 (A) / +12 (B)
                dst_a = bass.AP(
                    tensor=ys_ap.tensor, offset=base_off,
                    ap=[[c_step, C_OUT], [W * D, 2], [2 * WT * D, 4],
                        [D, WT], [1, D]])
                nc.sync.dma_start(
                    out=dst_a,
                    in_=stg[0:64, :].rearrange(
                        "p (r t w d) -> p r t w d", r=2, t=4, d=D))
                dst_b = bass.AP(
                    tensor=ys_ap.tensor, offset=base_off + WT * D,
                    ap=[[c_step, C_OUT], [W * D, 2], [2 * WT * D, 4],
                        [D, WT], [1, D]])
                nc.sync.dma_start(
                    out=dst_b,
                    in_=stg[64:128, :].rearrange(
                        "p (r t w d) -> p r t w d", r=2, t=4, d=D))

    nc.finalize()
    return nc


def _get_nc():
    if "nc" not in _CACHE:
        _CACHE["nc"] = _build()
    return _CACHE["nc"]


def _prep(x, w, gamma, beta):
    xpad = np.zeros((B, C_IN, H + 2, WP, DP), dtype=np.float32)
    xpad[:, :, 1:H + 1, 1:W + 1, 1:D + 1] = x
    wt = np.ascontiguousarray(
        np.asarray(w, dtype=np.float32).transpose(2, 3, 4, 1, 0).reshape(
            3, 3, 96, C_OUT)).astype(NP_BF16)
    gm = np.ascontiguousarray(np.asarray(gamma, dtype=np.float32))
    bt = np.ascontiguousarray(np.asarray(beta, dtype=np.float32))
    in_maps = []
    for c in range(N_CORES):
        xsl = np.ascontiguousarray(
            xpad[:, :, c * HS:c * HS + HR, :, :]).astype(NP_BF16)
        in_maps.append({"xs": xsl, "wt": wt, "gm": gm, "bt": bt})
    return in_maps


def kernel(x, w, b, gamma, beta):
    nc = _get_nc()
    in_maps = _prep(np.asarray(x, dtype=np.float32), w, gamma, beta)
    res = run_bass_kernel_spmd(nc, in_maps, core_ids=list(range(N_CORES)))
    out = np.concatenate([res.results[c]["ys"] for c in range(N_CORES)], axis=2)
    return out.astype(np.float32)


# revision 13
# speedup vs baseline: 1.1015x; 1.0039x over previous
"""Conv3d(32->64, k=3, pad=1) + BatchNorm(training) + LeakyReLU(0.2) on
(2, 32, 96, 96, 35), distributed over 8 TRN2 NeuronCores.

Strategy:
  - Shard H (96 = 8 x 12 rows per core). Halo rows + spatial zero-padding are
    materialized host-side into a per-core tensor xs of shape (2,32,14,98,39)
    (1 zero row each side of the 12-row H shard; W padded 96->98; D padded
    35->39 so that three d-shifted SBUF copies can be loaded by shifted reads).
  - Conv as implicit GEMM in bf16 (f32 PSUM accumulate): contraction
    K = 96 = C_in(32) x kd(3).  The SBUF "slab" for one input row holds 3
    partition-groups, group j pre-shifted by j elements along D.  Each of the
    9 (kh,kw) taps is then one matmul whose rhs is a free-dim-shifted window
    of the slab; kd is folded into the contraction.  PSUM accumulates taps.
  - M = C_out = 64 uses half the PE columns, so two spatial w-tiles run as a
    column-pair: tile A -> psum[0:64], tile B -> psum[64:128] (distinct PE
    column groups overlap in hardware).
  - W is tiled 96 = 8 x 12; a matmul streams 12w x 37d = 444 columns (the 2
    padded d columns per w are computed but never evicted).
  - BatchNorm (training stats): bn_stats per evicted tile -> bn_aggr ->
    per-partition (mean, var) -> (sum, sumsq) -> tiny AllReduce over the 8
    cores -> scale/shift folded into one parametric-relu activation.  The
    conv bias b cancels exactly in training-mode BN and is unused.  Conv
    values are parked in SBUF as bf16.
  - DMA: input slab loads ride the SP HWDGE ring, output stores the ACT
    HWDGE ring, so transfers overlap.
"""

import numpy as np
import ml_dtypes

import concourse.bacc as bacc
import concourse.bass as bass
import concourse.tile as tile
from concourse import mybir
from concourse.bass_utils import run_bass_kernel_spmd

N_CORES = 8
B, C_IN, C_OUT = 2, 32, 64
H, W, D = 96, 96, 35
HS = H // N_CORES          # 12 output rows per core
HR = HS + 2                # 14 input rows (halo)
WP, DP = W + 2, D + 2      # padded W / padded D for the host tensor
RW = D + 2                 # 37: slab row width per w-column (full padded D)
SLAB = WP * RW + 2 + 30    # slab row extent incl. group-shift + junk-read slack
WT = 12                    # w-tile width (8 uniform tiles)
NFULL = WT * RW            # 468 matmul free size
EVF = WT * D               # 420 evicted columns per tile
BLK = B * HS               # 24 (b,h) blocks per core
BLKCOLS = 4 * EVF          # 1680 conv-buffer columns per block per half
NREC = BLK * 4             # 96 bn_stats records per partition
CNT = float(BLK * 4 * EVF)          # elements per partition (40320)
N_TOT = float(B * H * W * D)        # 645120
EPS = 1e-5
NEG = 0.2

F32 = mybir.dt.float32
BF16 = mybir.dt.bfloat16
NP_BF16 = ml_dtypes.bfloat16

_CACHE = {}


def _build():
    nc = bacc.Bacc("TRN2", target_bir_lowering=False, debug=False,
                   num_devices=N_CORES)
    xs = nc.dram_tensor("xs", [B, C_IN, HR, WP, DP], BF16, kind="ExternalInput")
    wt = nc.dram_tensor("wt", [3, 3, 96, C_OUT], BF16, kind="ExternalInput")
    gm = nc.dram_tensor("gm", [C_OUT], F32, kind="ExternalInput")
    bt = nc.dram_tensor("bt", [C_OUT], F32, kind="ExternalInput")
    ys = nc.dram_tensor("ys", [B, C_OUT, HS, W, D], F32, kind="ExternalOutput")

    xs_ap = xs.ap()
    ys_ap = ys.ap()

    from contextlib import ExitStack
    with tile.TileContext(nc) as tc:
        with tc.tile_pool(name="singles", bufs=1) as singles, \
             tc.tile_pool(name="dram", bufs=1, space="DRAM") as dramp:
            phase1 = ExitStack()
            slabp = phase1.enter_context(tc.tile_pool(name="slab", bufs=4))
            psump = phase1.enter_context(
                tc.tile_pool(name="psum", bufs=4, space="PSUM"))

            # ---- one-time loads ----
            wtile = singles.tile([96, 9, C_OUT], BF16)
            nc.sync.dma_start(
                out=wtile,
                in_=wt.ap().rearrange("kh kw p o -> p (kh kw) o"))
            gmt = singles.tile([C_OUT, 1], F32)
            nc.sync.dma_start(out=gmt, in_=gm.ap().rearrange("(p o) -> p o", o=1))
            btt = singles.tile([C_OUT, 1], F32)
            nc.sync.dma_start(out=btt, in_=bt.ap().rearrange("(p o) -> p o", o=1))

            cb = singles.tile([128, BLK * BLKCOLS], BF16)   # conv results
            st = singles.tile([128, NREC * 6], F32)          # bn_stats records

            # ---- pass 1: conv + stats ----
            # slab group tiles hold 2 input rows (rows 2g, 2g+1)
            for b in range(B):
                groups = {}

                def need_rows(lo, hi, b=b, groups=None):
                    pass

                for h in range(HS):
                    for r in (h, h + 1, h + 2):
                        g = r // 2
                        if g not in groups:
                            gt = slabp.tile([96, 2, SLAB], BF16, tag="slab")
                            for j in range(3):
                                # group j holds the full row shifted by (2-j)
                                nc.sync.dma_start(
                                    out=gt[32 * j:32 * (j + 1), :,
                                           2 - j:2 - j + WP * RW],
                                    in_=xs_ap[b, :, 2 * g:2 * g + 2, :, :].rearrange(
                                        "p r w d -> p r (w d)"))
                            groups[g] = gt
                    blk = b * HS + h
                    for k in range(4):
                        w0a = 2 * k * WT
                        w0b = (2 * k + 1) * WT
                        ps = psump.tile([128, NFULL], F32, tag="ps")
                        for kh in range(3):
                            r = h + kh
                            gt = groups[r // 2]
                            rs = r % 2
                            for kw in range(3):
                                q = kh * 3 + kw
                                first, last = q == 0, q == 8
                                oa = (w0a + kw) * RW + 2
                                ob = (w0b + kw) * RW + 2
                                nc.tensor.matmul(
                                    ps[0:64, :],
                                    lhsT=wtile[:, q, :],
                                    rhs=gt[:, rs, oa:oa + NFULL],
                                    start=first, stop=last)
                                nc.tensor.matmul(
                                    ps[64:128, :],
                                    lhsT=wtile[:, q, :],
                                    rhs=gt[:, rs, ob:ob + NFULL],
                                    start=first, stop=last)
                        # evict + stats
                        col = blk * BLKCOLS + k * EVF
                        rec = (blk * 4 + k) * 6
                        pv = ps.rearrange("p (w d) -> p w d", d=RW)[:, :, 0:D]
                        nc.scalar.copy(
                            out=cb[:, col:col + EVF].rearrange(
                                "p (w d) -> p w d", d=D),
                            in_=pv)
                        nc.vector.bn_stats(out=st[:, rec:rec + 6],
                                           in_=cb[:, col:col + EVF])

            phase1.close()

            # ---- stats aggregation + allreduce ----
            mv = singles.tile([128, 2], F32)
            nc.vector.bn_aggr(out=mv, in_=st.rearrange("p (r s) -> p r s", s=6))
            sq = singles.tile([128, 2], F32)
            t1 = singles.tile([128, 1], F32)
            # sum = mean * n ; sumsq = (var + mean^2) * n
            nc.vector.tensor_scalar_mul(sq[:, 0:1], mv[:, 0:1], CNT)
            nc.vector.tensor_mul(t1, mv[:, 0:1], mv[:, 0:1])
            nc.vector.tensor_add(t1, t1, mv[:, 1:2])
            nc.vector.tensor_scalar_mul(sq[:, 1:2], t1, CNT)

            cc_in = dramp.tile([128, 2], F32)
            cc_out = dramp.tile([128, 2], F32)
            nc.sync.dma_start(out=cc_in[:, :], in_=sq)
            nc.gpsimd.collective_compute(
                "AllReduce", mybir.AluOpType.add,
                replica_groups=[list(range(N_CORES))],
                ins=[cc_in[:, :].opt()], outs=[cc_out[:, :].opt()])
            gl = singles.tile([128, 2], F32)
            nc.sync.dma_start(out=gl, in_=cc_out[:, :])

            hi = singles.tile([64, 2], F32)
            nc.sync.dma_start(out=hi, in_=gl[64:128, :])
            tot = singles.tile([64, 2], F32)
            nc.vector.tensor_add(tot, gl[0:64, :], hi)

            m_g = singles.tile([64, 1], F32)
            qn = singles.tile([64, 1], F32)
            var = singles.tile([64, 1], F32)
            sd = singles.tile([64, 1], F32)
            s64 = singles.tile([64, 1], F32)
            t64 = singles.tile([64, 1], F32)
            nc.vector.tensor_scalar_mul(m_g, tot[:, 0:1], 1.0 / N_TOT)
            nc.vector.tensor_scalar_mul(qn, tot[:, 1:2], 1.0 / N_TOT)
            nc.vector.tensor_mul(var, m_g, m_g)
            nc.vector.tensor_sub(var, qn, var)
            epst = singles.tile([64, 1], F32)
            nc.vector.memset(epst, EPS)
            nc.scalar.activation(out=sd, in_=var,
                                 func=mybir.ActivationFunctionType.Sqrt,
                                 bias=epst)
            nc.vector.reciprocal(out=sd, in_=sd)
            nc.vector.tensor_mul(s64, sd, gmt)      # s = gamma * rsqrt(var+eps)
            nc.vector.tensor_mul(t64, m_g, s64)
            nc.vector.tensor_sub(t64, btt, t64)     # t = beta - mean * s

            s_all = singles.tile([128, 1], F32)
            t_all = singles.tile([128, 1], F32)
            nc.vector.tensor_copy(s_all[0:64, :], s64)
            nc.vector.tensor_copy(t_all[0:64, :], t64)
            nc.sync.dma_start(out=s_all[64:128, :], in_=s_all[0:64, :])
            nc.sync.dma_start(out=t_all[64:128, :], in_=t_all[0:64, :])

            # ---- pass 2: normalize + LeakyReLU + writeback ----
            stgp = phase1.enter_context(tc.tile_pool(name="stg", bufs=6))
            c_step = HS * W * D  # ys channel stride
            for blk in range(0, BLK, 2):
                b_, h_ = divmod(blk, HS)
                stg = stgp.tile([128, 2 * BLKCOLS], F32, tag="stg")
                nc.scalar.activation(
                    out=stg, in_=cb[:, blk * BLKCOLS:(blk + 2) * BLKCOLS],
                    func=mybir.ActivationFunctionType.Prelu,
                    bias=t_all, scale=s_all, alpha=NEG)
                base_off = ys_ap.offset + b_ * (C_OUT * c_step) + h_ * (W * D)
                # two consecutive h rows; w-tiles at w = 0,24,48,72 (A) / +12 (B)
                dst_a = bass.AP(
                    tensor=ys_ap.tensor, offset=base_off,
                    ap=[[c_step, C_OUT], [W * D, 2], [2 * WT * D, 4],
                        [D, WT], [1, D]])
                nc.sync.dma_start(
                    out=dst_a,
                    in_=stg[0:64, :].rearrange(
                        "p (r t w d) -> p r t w d", r=2, t=4, d=D))
                dst_b = bass.AP(
                    tensor=ys_ap.tensor, offset=base_off + WT * D,
                    ap=[[c_step, C_OUT], [W * D, 2], [2 * WT * D, 4],
                        [D, WT], [1, D]])
                nc.scalar.dma_start(
                    out=dst_b,
                    in_=stg[64:128, :].rearrange(
                        "p (r t w d) -> p r t w d", r=2, t=4, d=D))

            phase1.close()
    nc.finalize()
    return nc


def _get_nc():
    if "nc" not in _CACHE:
        _CACHE["nc"] = _build()
    return _CACHE["nc"]


def _prep(x, w, gamma, beta):
    xpad = np.zeros((B, C_IN, H + 2, WP, DP), dtype=np.float32)
    xpad[:, :, 1:H + 1, 1:W + 1, 1:D + 1] = x
    wt = np.ascontiguousarray(
        np.asarray(w, dtype=np.float32).transpose(2, 3, 4, 1, 0).reshape(
            3, 3, 96, C_OUT)).astype(NP_BF16)
    gm = np.ascontiguousarray(np.asarray(gamma, dtype=np.float32))
    bt = np.ascontiguousarray(np.asarray(beta, dtype=np.float32))
    in_maps = []
    for c in range(N_CORES):
        xsl = np.ascontiguousarray(
            xpad[:, :, c * HS:c * HS + HR, :, :]).astype(NP_BF16)
        in_maps.append({"xs": xsl, "wt": wt, "gm": gm, "bt": bt})
    return in_maps


def kernel(x, w, b, gamma, beta):
    nc = _get_nc()
    in_maps = _prep(np.asarray(x, dtype=np.float32), w, gamma, beta)
    res = run_bass_kernel_spmd(nc, in_maps, core_ids=list(range(N_CORES)))
    out = np.concatenate([res.results[c]["ys"] for c in range(N_CORES)], axis=2)
    return out.astype(np.float32)
